# revision 19
# baseline (speedup 1.0000x reference)
"""Multi-head self-attention Trainium2 kernel (8 NeuronCores).

Problem: B=4, S=2048, K=128 head_dim, H=8 heads, fp32.
Sharding: batch*head-group parallel - core i computes batch b=i//2 and the
4 heads hg=i%2 (heads hg*4..hg*4+3), producing a partial output
y_part[b] = sum_{h in group} softmax(x M_h x^T) x W2_h.  The host adds the
two partials per batch plus bias.

Weight fusion (host-side, exact algebra):
  scores_h = x (Wq_h Wk_h^T * K^-.5) x^T  ->  M_h  := Wq_h Wk_h^T * K^-.5
  attnout_h @ Wo_h = (P_h x) (Wv_h Wo_h)  ->  W2_h := Wv_h Wo_h
so the device never materializes q/k/v: per head it computes
  qM = M_h^T-stationary over xT            [c, S]
  scoresT = xT-chunk-stationary over qM    [k, q]  (PSUM, pairs of 2x128 k)
  ex = exp(scoresT)                        bf16 (Act engine; some pairs on
                                           DVE via one-pass Schraudolph
                                           int16 bit-trick exp)
  G += x-chunk-stationary(bf16) over ex    [c, q]  (PSUM accumulate)
  den = column sums of ex                  (tiny bf16 PE folds -> dall)
  Gn = G * (1/den broadcast)               (DVE mult, f32r)
  yT += W2_h-stationary over Gn            (SBUF fp32 accumulate over heads)

The denominator transpose ([128 q-part, 8] -> [1, 512] row) is done with 4
partition-gather DMAs + one stride-0 broadcast DMA (no PE/DVE work).

All 16 (head, q-block) blocks run as ONE software-pipelined stream: the
G/denominator consumer lags the scores/exp producer by LAG k-pairs,
crossing block boundaries without refill bubbles.  Block epilogues and
next-head qM projections are closures drip-fed into the stream.
"""

import os

import numpy as np

WARM = int(os.environ.get("KERNEL_WARM", "3"))
LAG = int(os.environ.get("KERNEL_LAG", "5"))
TAPER = int(os.environ.get("KERNEL_TAPER", "3"))
SPACERS = int(os.environ.get("KERNEL_SPACERS", "3"))
# which pair indices (0..7) of every block run their exp on DVE instead of
# the Act engine (engine load balancing; numerics differ only by the
# Schraudolph approximation, validated ~8e-3 worst-case end to end)
DVE_PAIRS = tuple(
    int(t) for t in os.environ.get("KERNEL_DVE_PAIRS", "0,2,4,6").split(",")
    if t != ""
)

P = 128
S = 2048
NH = 4  # heads per core
SC = S // P  # 16 k-chunks
NP = SC // 2  # 8 k-pairs
NQ = S // 512  # 4 q-blocks per head
N_CORES = 8

# Schraudolph exp in bf16 bit-space: bf16(exp(x)) ~= bitcast16(A16*x + B16)
A16 = float(np.float32(2 ** 7 / np.log(2)))
B16 = float(np.float32(127 * 2 ** 7 - 366000.0 / 65536.0))

_CACHE = {}
LAST_RESULTS = None


def _tf32_round(a):
    """Round fp32 array to tf32 (10-bit mantissa) bit patterns, RNE."""
    bits = np.ascontiguousarray(a, dtype=np.float32).view(np.uint32)
    rounded = bits + np.uint32(0x0FFF) + ((bits >> np.uint32(13)) & np.uint32(1))
    rounded &= np.uint32(0xFFFFE000)
    return rounded.view(np.float32)


def _build():
    from contextlib import ExitStack

    import concourse.bass as bass
    import concourse.tile as tile
    from concourse import bacc, mybir

    f32 = mybir.dt.float32
    f32r = mybir.dt.float32r
    bf16 = mybir.dt.bfloat16
    i16 = mybir.dt.int16
    Exp = mybir.ActivationFunctionType.Exp

    nc = bacc.Bacc("TRN2", target_bir_lowering=False, debug=False,
                   num_devices=N_CORES)
    xt = nc.dram_tensor("xt", [P, S], f32r, kind="ExternalInput").ap()
    xb = nc.dram_tensor("xb", [P, S], bf16, kind="ExternalInput").ap()
    wm = nc.dram_tensor("wm", [P, NH * P], f32r, kind="ExternalInput").ap()
    w2 = nc.dram_tensor("w2", [P, NH * P], f32r, kind="ExternalInput").ap()
    y = nc.dram_tensor("y", [P, S], f32, kind="ExternalOutput").ap()

    with tile.TileContext(nc) as tc, ExitStack() as ctx:
        consts = ctx.enter_context(tc.tile_pool(name="consts", bufs=1))
        bigs = ctx.enter_context(tc.tile_pool(name="bigs", bufs=1))
        qmp = ctx.enter_context(tc.tile_pool(name="qmp", bufs=2))
        expp = ctx.enter_context(tc.tile_pool(name="expp", bufs=8))
        outp = ctx.enter_context(tc.tile_pool(name="outp", bufs=2))
        small = ctx.enter_context(tc.tile_pool(name="small", bufs=4))
        # PSUM banks: stage 2x[128,1024]=4 + G accum 2x[128,512]=2
        #             + dall 1 + psy/prefetch 1  -> 8 banks
        psStage = ctx.enter_context(tc.tile_pool(name="psStage", bufs=2, space="PSUM"))
        psOut = ctx.enter_context(tc.tile_pool(name="psOut", bufs=2, space="PSUM"))
        psAux = ctx.enter_context(tc.tile_pool(name="psAux", bufs=1, space="PSUM"))
        psDall = ctx.enter_context(tc.tile_pool(name="psDall", bufs=1, space="PSUM"))

        # --- DMAs on the SP queue, first-needed-first ---
        xT = bigs.tile([P, S], f32r)
        xB = bigs.tile([P, S], bf16)
        wm_r = consts.tile([P, NH * P], f32r)
        w2_r = consts.tile([P, NH * P], f32r)
        nc.sync.dma_start(xT[:, 0:512], xt[:, 0:512])
        nc.sync.dma_start(wm_r[:, 0:P], wm[:, 0:P])
        nc.sync.dma_start(xT[:, 512:2048], xt[:, 512:2048])
        nc.sync.dma_start(xB[:], xb[:])
        nc.sync.dma_start(wm_r[:, P:], wm[:, P:])
        nc.sync.dma_start(w2_r[:], w2[:])

        # ONE psum bank shared by the denominator folds (cols 0:8, one
        # accumulation group per block) and the PE warm-up target (row 0,
        # temporally disjoint)
        dall = psDall.tile([P, 512], f32, name="dall", tag="dall")
        ones_col = consts.tile([P, 1], f32r)
        nc.vector.memset(ones_col.bitcast(f32)[:], 1.0)
        warm_r = consts.tile([P, 512], f32r)
        nc.vector.memset(warm_r.bitcast(f32)[:], 1.0)
        # pre-heat the PE during the DMA dead zone: dummy matmuls release
        # the HAM clock gate (1.2 -> 2.4 GHz) before real work lands
        for i in range(WARM):
            nc.tensor.matmul(dall[0:1, :], ones_col[:], warm_r[:],
                             start=True, stop=True)
        # [128,2] bf16 ones: moving operand of the tiny denominator folds
        # (the ISA rejects 1-column moving operands)
        ones_b2 = consts.tile([P, 2], bf16)
        nc.gpsimd.memset(ones_b2[:], 1.0)
        ones_row_f = consts.tile([1, P], f32)
        nc.gpsimd.memset(ones_row_f[:], 1.0)
        # preload the Exp activation table during the DMA dead zone so the
        # first real exp doesn't eat the ~1.3us table load
        act_scr = consts.tile([1, 2], f32)
        nc.scalar.activation(act_scr[:], ones_row_f[0:1, 0:2], Exp)

        yT = bigs.tile([P, S], f32)

        heads_qm = {0: qmp.tile([P, S], f32r, name="qM0", tag="qM")}

        def qm_step(h, qc, pool, tag, eng=None):
            """qM_h[:, qc*512:...] = wm_h-stationary matmul over xT chunk.
            Drains ride the Act engine by default (Copy shares the Exp
            activation table, so no mid-stream table reloads)."""
            ps = pool.tile([P, 512], f32, name=f"qm{h}{qc}", tag=tag)
            nc.tensor.matmul(ps[:], wm_r[:, h * P:(h + 1) * P],
                             xT[:, qc * 512:(qc + 1) * 512],
                             start=True, stop=True)
            (eng or nc.scalar.copy)(heads_qm[h][:, qc * 512:(qc + 1) * 512],
                                    ps[:])

        # critical path: only qM(h0, chunk0) gates the first scores pair
        qm_step(0, 0, psStage, "stage")
        startup_bg = [lambda qc=qc: qm_step(0, qc, psAux, "aux")
                      for qc in (1, 2, 3)]

        # --- main stream over 16 blocks, iterating k-pairs ---
        blocks = [(h, qcb) for h in range(NH) for qcb in range(NQ)]
        NB = len(blocks)
        bstate = {}
        bg_epi = []
        bg_pre = []

        def make_prefetch(h):
            heads_qm[h] = qmp.tile([P, S], f32r, name=f"qM{h}", tag="qM")
            return [lambda h=h, qc=qc: qm_step(h, qc, psAux, "aux")
                    for qc in range(4)]

        def make_epilogue(h, qcb, bs):
            tail = h == NH - 1 and qcb == NQ - 1
            if tail:
                # kernel tail: split into two 256-wide chains; broadcast via
                # a rank-1 PE matmul instead of the DMA round trip
                chains = [_make_epi_part(h, qcb, bs, off, 256, tail=True)
                          for off in (0, 256)]
                return [s for pair in zip(*chains) for s in pair]
            # spacers delay the norm so its bcs DMA chain (gather ->
            # broadcast, ~2.5us) completes before norm reaches the head of
            # the in-order DVE queue, where a stalled wait would block the
            # next block's DVE exp
            steps = _make_epi_part(h, qcb, bs, 0, 512, tail=False)
            return steps[:1] + [(lambda: None)] * SPACERS + steps[1:]

        def _make_epi_part(h, qcb, bs, off, w, tail):
            q0 = qcb * 512 + off
            steps = []
            bcs = small.tile([P, w], f32, name=f"bcs{h}{qcb}{off}",
                             tag="bc_sb")
            rec8, rrow = bs["rec8"], bs["rrow"]

            def bcast():
                if tail:
                    # gather the 2 (j) columns for this half on PE+Act:
                    # rank-1 f32 broadcast matmuls from rrow
                    bc = psStage.tile([P, w], f32, name=f"bc{h}{qcb}{off}",
                                      tag="stage")
                    nc.tensor.matmul(bc[:], ones_row_f[:],
                                     rrow[0:1, off:off + w],
                                     start=True, stop=True)
                    nc.scalar.copy(bcs[:], bc[:])
                else:
                    nc.sync.dma_start(
                        bcs[:],
                        rrow[0:1, off:off + w]
                        .rearrange("(a b) w -> a b w", b=1)
                        .broadcast_to((1, P, w)))
            steps.append(bcast)

            def norm():
                nc.vector.tensor_mul(bs["outTn"][:, q0:q0 + w],
                                     bs["outPS"][:, off:off + w], bcs[:])
            steps.append(norm)

            def yacc():
                psy = psAux.tile([P, w], f32, name=f"psy{h}{qcb}{off}",
                                 tag="aux")
                nc.tensor.matmul(psy[:], w2_r[:, h * P:(h + 1) * P],
                                 bs["outTn"][:, q0:q0 + w],
                                 start=True, stop=True)
                if h == 0:
                    nc.vector.tensor_copy(yT[:, q0:q0 + w], psy[:])
                else:
                    nc.vector.tensor_add(yT[:, q0:q0 + w],
                                         yT[:, q0:q0 + w], psy[:])
                if h == NH - 1:
                    nc.sync.dma_start(y[:, q0:q0 + w], yT[:, q0:q0 + w])
            steps.append(yacc)
            return steps

        # consumer lag: LAG mid-stream, tapering for the last block
        cons = 0
        total_pairs = NB * NP

        def lag_of(ci):
            return LAG if ci < (NB - 1) * NP else TAPER

        for j in range(NB * NP + LAG):
            if j < NB * NP:
                b, p = divmod(j, NP)
                h, qcb = blocks[b]
                if p == 0:
                    bs = bstate[b] = {
                        "qM": heads_qm[h],
                        "outTn": (bstate[b - 1]["outTn"]
                                  if qcb != 0 else
                                  outp.tile([P, S], f32r, name=f"outTn{h}",
                                            tag="outTn")),
                        "outPS": psOut.tile([P, 512], f32, name=f"oPS{h}{qcb}",
                                            tag="po"),
                        "exs": [None] * NP,
                    }
                    if qcb == NQ - 3 and h + 1 < NH:
                        bg_pre.extend(make_prefetch(h + 1))
                else:
                    bs = bstate[b]
                q0 = qcb * 512
                st = psStage.tile([P, 1024], f32, name=f"st{h}{qcb}{p}",
                                  tag="stage")
                for half in range(2):
                    kc = 2 * p + half
                    nc.tensor.matmul(st[:, half * 512:(half + 1) * 512],
                                     xT[:, kc * P:(kc + 1) * P],
                                     bs["qM"][:, q0:q0 + 512],
                                     start=True, stop=True)
                ex = expp.tile([P, 1024], bf16, name=f"ex{h}{qcb}{p}",
                               tag="exp")
                if p in DVE_PAIRS:
                    # one-pass Schraudolph exp on DVE: int16(A16*s + B16)
                    # lands the bf16 bit pattern of exp(s)
                    nc.vector.tensor_scalar(ex.bitcast(i16)[:], st[:],
                                            A16, B16,
                                            mybir.AluOpType.mult,
                                            mybir.AluOpType.add)
                else:
                    nc.scalar.activation(ex[:], st[:], Exp)
                bs["exs"][p] = ex
            while cons < total_pairs and cons <= j - lag_of(cons):
                jj = cons
                cons += 1
                b2, p2 = divmod(jj, NP)
                h2, qcb2 = blocks[b2]
                bs2 = bstate[b2]
                exp_pair = bs2["exs"][p2]
                for half in range(2):
                    k2 = 2 * p2 + half
                    exh = exp_pair[:, half * 512:(half + 1) * 512]
                    xbh = xB[:, k2 * P:(k2 + 1) * P]
                    nc.tensor.matmul(bs2["outPS"][:], xbh, exh,
                                     start=(k2 == 0), stop=(k2 == SC - 1))
                # denominator: 8 tiny bf16 fold matmuls per pair with a
                # STRIDED slice of the exp tile as STATIONARY and a bf16
                # ones column pair moving; dall[q//4, 2*(q%4)] accumulates
                # sum_k ex over the block's 16 k-chunks.  The q%4-interleaved
                # layout lets ONE partition-gather DMA rebuild the [1,512]
                # denominator row in q order.
                for half in range(2):
                    for jq in range(4):
                        c0 = half * 512
                        nc.tensor.matmul(dall[:, 2 * jq:2 * jq + 2],
                                         exp_pair[:, c0 + jq:c0 + 512:4],
                                         ones_b2[:],
                                         start=(p2 == 0 and half == 0
                                                and jq == 0),
                                         stop=(p2 == NP - 1 and half == 1
                                               and jq == 3))
                if p2 == NP - 1:
                    # reciprocal straight off the psum fold accumulators NOW
                    # (inline) so the next block's folds can't clobber them
                    rec8 = small.tile([P, 8], f32, name=f"r8{h2}{qcb2}",
                                      tag="rec8")
                    nc.vector.reciprocal_approx_fast(rec8[:], dall[:, 0:8])
                    rrow = small.tile([1, 512], f32, name=f"rr{h2}{qcb2}",
                                      tag="rrow")
                    bs2["rec8"], bs2["rrow"] = rec8, rrow

                    def gather(rec8=rec8, rrow=rrow):
                        # rrow[0, 4*r + jq] = rec8[r, 2*jq]
                        nc.sync.dma_start(rrow[0:1, :], rec8[:, 0:8:2])
                    bg_epi.append(gather)
                    bg_epi.extend(make_epilogue(h2, qcb2, bs2))
                    bstate.pop(b2 - 1, None)
            # drip-feed background work, epilogues first (release PSUM
            # slots); nothing drips on block-start iterations (p == 0) so
            # the next block's scores aren't queued behind background
            # matmuls on the in-order PE
            n_su = 1 if j == 0 else 2
            for _ in range(n_su):
                if startup_bg:
                    startup_bg.pop(0)()
            if bg_epi:
                bg_epi.pop(0)()
            if bg_pre and j % 3 == 2 and j < (NB - 1) * NP:
                bg_pre.pop(0)()
            elif bg_epi:
                bg_epi.pop(0)()
        while bg_epi or bg_pre:
            (bg_epi or bg_pre).pop(0)()

    nc.compile()
    return nc


def _get_nc():
    if "nc" not in _CACHE:
        _CACHE["nc"] = _build()
    return _CACHE["nc"]


def kernel(x, Wq, Wk, Wv, Wo, bo):
    global LAST_RESULTS
    import ml_dtypes
    from concourse.bass_utils import run_bass_kernel_spmd

    x = np.asarray(x, dtype=np.float32)
    Wq = np.asarray(Wq, dtype=np.float32)
    Wk = np.asarray(Wk, dtype=np.float32)
    Wv = np.asarray(Wv, dtype=np.float32)
    Wo = np.asarray(Wo, dtype=np.float32)
    bo = np.asarray(bo, dtype=np.float32)

    nc = _get_nc()
    qk_scale = np.float32(P ** -0.5)
    in_maps = []
    for core in range(N_CORES):
        b, hg = core // 2, core % 2
        heads = [hg * NH + h for h in range(NH)]
        # fused weights: M_h = Wq_h Wk_h^T * K^-.5 (scores, [c', c]),
        #                W2_h = Wv_h Wo_h (output, [c, dout])
        wm_np = np.concatenate(
            [Wq[:, h * P:(h + 1) * P] @ Wk[:, h * P:(h + 1) * P].T * qk_scale
             for h in heads], axis=1)
        w2_np = np.concatenate(
            [Wv[:, h * P:(h + 1) * P] @ Wo[h * P:(h + 1) * P, :]
             for h in heads], axis=1)
        in_maps.append({
            "xt": _tf32_round(x[b].T),
            # x chunk-major for the G stationary: xb[p, t*128+c] = x[t*128+p, c]
            "xb": np.ascontiguousarray(
                x[b].reshape(SC, P, P).transpose(1, 0, 2).reshape(P, S)
            ).astype(ml_dtypes.bfloat16),
            "wm": _tf32_round(wm_np),
            "w2": _tf32_round(w2_np),
        })
    trace = bool(int(os.environ.get("KERNEL_TRACE", "0")))
    res = run_bass_kernel_spmd(nc, in_maps, core_ids=list(range(N_CORES)),
                               trace=trace)
    LAST_RESULTS = res
    parts = [np.ascontiguousarray(r["y"].T) for r in res.results]
    out = np.stack([parts[2 * b] + parts[2 * b + 1] + bo[None, :]
                    for b in range(4)])
    return out.astype(np.float32)


# revision 20
# speedup vs baseline: 1.0010x; 1.0010x over previous
"""Multi-head self-attention Trainium2 kernel (8 NeuronCores).

Problem: B=4, S=2048, K=128 head_dim, H=8 heads, fp32.
Sharding: batch*head-group parallel - core i computes batch b=i//2 and the
4 heads hg=i%2 (heads hg*4..hg*4+3), producing a partial output
y_part[b] = sum_{h in group} softmax(x M_h x^T) x W2_h.  The host adds the
two partials per batch plus bias.

Weight fusion (host-side, exact algebra):
  scores_h = x (Wq_h Wk_h^T * K^-.5) x^T  ->  M_h  := Wq_h Wk_h^T * K^-.5
  attnout_h @ Wo_h = (P_h x) (Wv_h Wo_h)  ->  W2_h := Wv_h Wo_h
so the device never materializes q/k/v: per head it computes
  qM = M_h^T-stationary over xT            [c, S]
  scoresT = xT-chunk-stationary over qM    [k, q]  (PSUM, pairs of 2x128 k)
  ex = exp(scoresT)                        bf16 (Act engine; some pairs on
                                           DVE via one-pass Schraudolph
                                           int16 bit-trick exp)
  G += x-chunk-stationary(bf16) over ex    [c, q]  (PSUM accumulate)
  den = column sums of ex                  (tiny bf16 PE folds -> dall)
  Gn = G * (1/den broadcast)               (DVE mult, f32r)
  yT += W2_h-stationary over Gn            (SBUF fp32 accumulate over heads)

The denominator transpose ([128 q-part, 8] -> [1, 512] row) is done with 4
partition-gather DMAs + one stride-0 broadcast DMA (no PE/DVE work).

All 16 (head, q-block) blocks run as ONE software-pipelined stream: the
G/denominator consumer lags the scores/exp producer by LAG k-pairs,
crossing block boundaries without refill bubbles.  Block epilogues and
next-head qM projections are closures drip-fed into the stream.
"""

import os

import numpy as np

WARM = int(os.environ.get("KERNEL_WARM", "3"))
LAG = int(os.environ.get("KERNEL_LAG", "5"))
TAPER = int(os.environ.get("KERNEL_TAPER", "3"))
SPACERS = int(os.environ.get("KERNEL_SPACERS", "2"))
# which pair indices (0..7) of every block run their exp on DVE instead of
# the Act engine (engine load balancing; numerics differ only by the
# Schraudolph approximation, validated ~8e-3 worst-case end to end)
DVE_PAIRS = tuple(
    int(t) for t in os.environ.get("KERNEL_DVE_PAIRS", "0,2,4,6").split(",")
    if t != ""
)

P = 128
S = 2048
NH = 4  # heads per core
SC = S // P  # 16 k-chunks
NP = SC // 2  # 8 k-pairs
NQ = S // 512  # 4 q-blocks per head
N_CORES = 8

# Schraudolph exp in bf16 bit-space: bf16(exp(x)) ~= bitcast16(A16*x + B16)
A16 = float(np.float32(2 ** 7 / np.log(2)))
B16 = float(np.float32(127 * 2 ** 7 - 366000.0 / 65536.0))

_CACHE = {}
LAST_RESULTS = None


def _tf32_round(a):
    """Round fp32 array to tf32 (10-bit mantissa) bit patterns, RNE."""
    bits = np.ascontiguousarray(a, dtype=np.float32).view(np.uint32)
    rounded = bits + np.uint32(0x0FFF) + ((bits >> np.uint32(13)) & np.uint32(1))
    rounded &= np.uint32(0xFFFFE000)
    return rounded.view(np.float32)


def _build():
    from contextlib import ExitStack

    import concourse.bass as bass
    import concourse.tile as tile
    from concourse import bacc, mybir

    f32 = mybir.dt.float32
    f32r = mybir.dt.float32r
    bf16 = mybir.dt.bfloat16
    i16 = mybir.dt.int16
    Exp = mybir.ActivationFunctionType.Exp

    nc = bacc.Bacc("TRN2", target_bir_lowering=False, debug=False,
                   num_devices=N_CORES)
    xt = nc.dram_tensor("xt", [P, S], f32r, kind="ExternalInput").ap()
    xb = nc.dram_tensor("xb", [P, S], bf16, kind="ExternalInput").ap()
    wm = nc.dram_tensor("wm", [P, NH * P], f32r, kind="ExternalInput").ap()
    w2 = nc.dram_tensor("w2", [P, NH * P], f32r, kind="ExternalInput").ap()
    y = nc.dram_tensor("y", [P, S], f32, kind="ExternalOutput").ap()

    with tile.TileContext(nc) as tc, ExitStack() as ctx:
        consts = ctx.enter_context(tc.tile_pool(name="consts", bufs=1))
        bigs = ctx.enter_context(tc.tile_pool(name="bigs", bufs=1))
        qmp = ctx.enter_context(tc.tile_pool(name="qmp", bufs=2))
        expp = ctx.enter_context(tc.tile_pool(name="expp", bufs=8))
        outp = ctx.enter_context(tc.tile_pool(name="outp", bufs=2))
        small = ctx.enter_context(tc.tile_pool(name="small", bufs=4))
        # PSUM banks: stage 2x[128,1024]=4 + G accum 2x[128,512]=2
        #             + dall 1 + psy/prefetch 1  -> 8 banks
        psStage = ctx.enter_context(tc.tile_pool(name="psStage", bufs=2, space="PSUM"))
        psOut = ctx.enter_context(tc.tile_pool(name="psOut", bufs=2, space="PSUM"))
        psAux = ctx.enter_context(tc.tile_pool(name="psAux", bufs=1, space="PSUM"))
        psDall = ctx.enter_context(tc.tile_pool(name="psDall", bufs=1, space="PSUM"))

        # --- DMAs on the SP queue, first-needed-first ---
        xT = bigs.tile([P, S], f32r)
        xB = bigs.tile([P, S], bf16)
        wm_r = consts.tile([P, NH * P], f32r)
        w2_r = consts.tile([P, NH * P], f32r)
        nc.sync.dma_start(xT[:, 0:512], xt[:, 0:512])
        nc.sync.dma_start(wm_r[:, 0:P], wm[:, 0:P])
        nc.sync.dma_start(xT[:, 512:2048], xt[:, 512:2048])
        nc.sync.dma_start(xB[:], xb[:])
        nc.sync.dma_start(wm_r[:, P:], wm[:, P:])
        nc.sync.dma_start(w2_r[:], w2[:])

        # ONE psum bank shared by the denominator folds (cols 0:8, one
        # accumulation group per block) and the PE warm-up target (row 0,
        # temporally disjoint)
        dall = psDall.tile([P, 512], f32, name="dall", tag="dall")
        ones_col = consts.tile([P, 1], f32r)
        nc.vector.memset(ones_col.bitcast(f32)[:], 1.0)
        warm_r = consts.tile([P, 512], f32r)
        nc.vector.memset(warm_r.bitcast(f32)[:], 1.0)
        # pre-heat the PE during the DMA dead zone: dummy matmuls release
        # the HAM clock gate (1.2 -> 2.4 GHz) before real work lands
        for i in range(WARM):
            nc.tensor.matmul(dall[0:1, :], ones_col[:], warm_r[:],
                             start=True, stop=True)
        # [128,2] bf16 ones: moving operand of the tiny denominator folds
        # (the ISA rejects 1-column moving operands)
        ones_b2 = consts.tile([P, 2], bf16)
        nc.gpsimd.memset(ones_b2[:], 1.0)
        ones_row_f = consts.tile([1, P], f32)
        nc.gpsimd.memset(ones_row_f[:], 1.0)
        # preload the Exp activation table during the DMA dead zone so the
        # first real exp doesn't eat the ~1.3us table load
        act_scr = consts.tile([1, 2], f32)
        nc.scalar.activation(act_scr[:], ones_row_f[0:1, 0:2], Exp)

        yT = bigs.tile([P, S], f32)

        heads_qm = {0: qmp.tile([P, S], f32r, name="qM0", tag="qM")}

        def qm_step(h, qc, pool, tag, eng=None):
            """qM_h[:, qc*512:...] = wm_h-stationary matmul over xT chunk.
            Drains ride the Act engine by default (Copy shares the Exp
            activation table, so no mid-stream table reloads)."""
            ps = pool.tile([P, 512], f32, name=f"qm{h}{qc}", tag=tag)
            nc.tensor.matmul(ps[:], wm_r[:, h * P:(h + 1) * P],
                             xT[:, qc * 512:(qc + 1) * 512],
                             start=True, stop=True)
            (eng or nc.scalar.copy)(heads_qm[h][:, qc * 512:(qc + 1) * 512],
                                    ps[:])

        # critical path: only qM(h0, chunk0) gates the first scores pair
        qm_step(0, 0, psStage, "stage")
        startup_bg = [lambda qc=qc: qm_step(0, qc, psAux, "aux")
                      for qc in (1, 2, 3)]

        # --- main stream over 16 blocks, iterating k-pairs ---
        blocks = [(h, qcb) for h in range(NH) for qcb in range(NQ)]
        NB = len(blocks)
        bstate = {}
        bg_epi = []
        bg_pre = []

        def make_prefetch(h):
            heads_qm[h] = qmp.tile([P, S], f32r, name=f"qM{h}", tag="qM")
            return [lambda h=h, qc=qc: qm_step(h, qc, psAux, "aux")
                    for qc in range(4)]

        def make_epilogue(h, qcb, bs):
            tail = h == NH - 1 and qcb == NQ - 1
            if tail:
                # kernel tail: split into two 256-wide chains; broadcast via
                # a rank-1 PE matmul instead of the DMA round trip
                chains = [_make_epi_part(h, qcb, bs, off, 256, tail=True)
                          for off in (0, 256)]
                return [s for pair in zip(*chains) for s in pair]
            # spacers delay the norm so its bcs DMA chain (gather ->
            # broadcast, ~2.5us) completes before norm reaches the head of
            # the in-order DVE queue, where a stalled wait would block the
            # next block's DVE exp
            steps = _make_epi_part(h, qcb, bs, 0, 512, tail=False)
            return steps[:1] + [(lambda: None)] * SPACERS + steps[1:]

        def _make_epi_part(h, qcb, bs, off, w, tail):
            q0 = qcb * 512 + off
            steps = []
            bcs = small.tile([P, w], f32, name=f"bcs{h}{qcb}{off}",
                             tag="bc_sb")
            rec8, rrow = bs["rec8"], bs["rrow"]

            def bcast():
                if tail:
                    # gather the 2 (j) columns for this half on PE+Act:
                    # rank-1 f32 broadcast matmuls from rrow
                    bc = psStage.tile([P, w], f32, name=f"bc{h}{qcb}{off}",
                                      tag="stage")
                    nc.tensor.matmul(bc[:], ones_row_f[:],
                                     rrow[0:1, off:off + w],
                                     start=True, stop=True)
                    nc.scalar.copy(bcs[:], bc[:])
                else:
                    nc.sync.dma_start(
                        bcs[:],
                        rrow[0:1, off:off + w]
                        .rearrange("(a b) w -> a b w", b=1)
                        .broadcast_to((1, P, w)))
            steps.append(bcast)

            def norm():
                nc.vector.tensor_mul(bs["outTn"][:, q0:q0 + w],
                                     bs["outPS"][:, off:off + w], bcs[:])
            steps.append(norm)

            def yacc():
                psy = psAux.tile([P, w], f32, name=f"psy{h}{qcb}{off}",
                                 tag="aux")
                nc.tensor.matmul(psy[:], w2_r[:, h * P:(h + 1) * P],
                                 bs["outTn"][:, q0:q0 + w],
                                 start=True, stop=True)
                if h == 0:
                    nc.vector.tensor_copy(yT[:, q0:q0 + w], psy[:])
                else:
                    nc.vector.tensor_add(yT[:, q0:q0 + w],
                                         yT[:, q0:q0 + w], psy[:])
                if h == NH - 1:
                    nc.sync.dma_start(y[:, q0:q0 + w], yT[:, q0:q0 + w])
            steps.append(yacc)
            return steps

        # consumer lag: LAG mid-stream, tapering for the last block
        cons = 0
        total_pairs = NB * NP

        def lag_of(ci):
            return LAG if ci < (NB - 1) * NP else TAPER

        for j in range(NB * NP + LAG):
            if j < NB * NP:
                b, p = divmod(j, NP)
                h, qcb = blocks[b]
                if p == 0:
                    bs = bstate[b] = {
                        "qM": heads_qm[h],
                        "outTn": (bstate[b - 1]["outTn"]
                                  if qcb != 0 else
                                  outp.tile([P, S], f32r, name=f"outTn{h}",
                                            tag="outTn")),
                        "outPS": psOut.tile([P, 512], f32, name=f"oPS{h}{qcb}",
                                            tag="po"),
                        "exs": [None] * NP,
                    }
                    if qcb == NQ - 3 and h + 1 < NH:
                        bg_pre.extend(make_prefetch(h + 1))
                else:
                    bs = bstate[b]
                q0 = qcb * 512
                st = psStage.tile([P, 1024], f32, name=f"st{h}{qcb}{p}",
                                  tag="stage")
                for half in range(2):
                    kc = 2 * p + half
                    nc.tensor.matmul(st[:, half * 512:(half + 1) * 512],
                                     xT[:, kc * P:(kc + 1) * P],
                                     bs["qM"][:, q0:q0 + 512],
                                     start=True, stop=True)
                ex = expp.tile([P, 1024], bf16, name=f"ex{h}{qcb}{p}",
                               tag="exp")
                if p in DVE_PAIRS:
                    # one-pass Schraudolph exp on DVE: int16(A16*s + B16)
                    # lands the bf16 bit pattern of exp(s)
                    nc.vector.tensor_scalar(ex.bitcast(i16)[:], st[:],
                                            A16, B16,
                                            mybir.AluOpType.mult,
                                            mybir.AluOpType.add)
                else:
                    nc.scalar.activation(ex[:], st[:], Exp)
                bs["exs"][p] = ex
            while cons < total_pairs and cons <= j - lag_of(cons):
                jj = cons
                cons += 1
                b2, p2 = divmod(jj, NP)
                h2, qcb2 = blocks[b2]
                bs2 = bstate[b2]
                exp_pair = bs2["exs"][p2]
                for half in range(2):
                    k2 = 2 * p2 + half
                    exh = exp_pair[:, half * 512:(half + 1) * 512]
                    xbh = xB[:, k2 * P:(k2 + 1) * P]
                    nc.tensor.matmul(bs2["outPS"][:], xbh, exh,
                                     start=(k2 == 0), stop=(k2 == SC - 1))
                # denominator: 8 tiny bf16 fold matmuls per pair with a
                # STRIDED slice of the exp tile as STATIONARY and a bf16
                # ones column pair moving; dall[q//4, 2*(q%4)] accumulates
                # sum_k ex over the block's 16 k-chunks.  The q%4-interleaved
                # layout lets ONE partition-gather DMA rebuild the [1,512]
                # denominator row in q order.
                for half in range(2):
                    for jq in range(4):
                        c0 = half * 512
                        nc.tensor.matmul(dall[:, 2 * jq:2 * jq + 2],
                                         exp_pair[:, c0 + jq:c0 + 512:4],
                                         ones_b2[:],
                                         start=(p2 == 0 and half == 0
                                                and jq == 0),
                                         stop=(p2 == NP - 1 and half == 1
                                               and jq == 3))
                if p2 == NP - 1:
                    # reciprocal straight off the psum fold accumulators NOW
                    # (inline) so the next block's folds can't clobber them
                    rec8 = small.tile([P, 8], f32, name=f"r8{h2}{qcb2}",
                                      tag="rec8")
                    nc.vector.reciprocal_approx_fast(rec8[:], dall[:, 0:8])
                    rrow = small.tile([1, 512], f32, name=f"rr{h2}{qcb2}",
                                      tag="rrow")
                    bs2["rec8"], bs2["rrow"] = rec8, rrow

                    def gather(rec8=rec8, rrow=rrow):
                        # rrow[0, 4*r + jq] = rec8[r, 2*jq]
                        nc.sync.dma_start(rrow[0:1, :], rec8[:, 0:8:2])
                    bg_epi.append(gather)
                    bg_epi.extend(make_epilogue(h2, qcb2, bs2))
                    bstate.pop(b2 - 1, None)
            # drip-feed background work, epilogues first (release PSUM
            # slots); nothing drips on block-start iterations (p == 0) so
            # the next block's scores aren't queued behind background
            # matmuls on the in-order PE
            n_su = 1 if j == 0 else 2
            for _ in range(n_su):
                if startup_bg:
                    startup_bg.pop(0)()
            if bg_epi:
                bg_epi.pop(0)()
            if bg_pre and j % 3 == 2 and j < (NB - 1) * NP:
                bg_pre.pop(0)()
            elif bg_epi:
                bg_epi.pop(0)()
        while bg_epi or bg_pre:
            (bg_epi or bg_pre).pop(0)()

    nc.compile()
    return nc


def _get_nc():
    if "nc" not in _CACHE:
        _CACHE["nc"] = _build()
    return _CACHE["nc"]


def kernel(x, Wq, Wk, Wv, Wo, bo):
    global LAST_RESULTS
    import ml_dtypes
    from concourse.bass_utils import run_bass_kernel_spmd

    x = np.asarray(x, dtype=np.float32)
    Wq = np.asarray(Wq, dtype=np.float32)
    Wk = np.asarray(Wk, dtype=np.float32)
    Wv = np.asarray(Wv, dtype=np.float32)
    Wo = np.asarray(Wo, dtype=np.float32)
    bo = np.asarray(bo, dtype=np.float32)

    nc = _get_nc()
    qk_scale = np.float32(P ** -0.5)
    in_maps = []
    for core in range(N_CORES):
        b, hg = core // 2, core % 2
        heads = [hg * NH + h for h in range(NH)]
        # fused weights: M_h = Wq_h Wk_h^T * K^-.5 (scores, [c', c]),
        #                W2_h = Wv_h Wo_h (output, [c, dout])
        wm_np = np.concatenate(
            [Wq[:, h * P:(h + 1) * P] @ Wk[:, h * P:(h + 1) * P].T * qk_scale
             for h in heads], axis=1)
        w2_np = np.concatenate(
            [Wv[:, h * P:(h + 1) * P] @ Wo[h * P:(h + 1) * P, :]
             for h in heads], axis=1)
        in_maps.append({
            "xt": _tf32_round(x[b].T),
            # x chunk-major for the G stationary: xb[p, t*128+c] = x[t*128+p, c]
            "xb": np.ascontiguousarray(
                x[b].reshape(SC, P, P).transpose(1, 0, 2).reshape(P, S)
            ).astype(ml_dtypes.bfloat16),
            "wm": _tf32_round(wm_np),
            "w2": _tf32_round(w2_np),
        })
    trace = bool(int(os.environ.get("KERNEL_TRACE", "0")))
    res = run_bass_kernel_spmd(nc, in_maps, core_ids=list(range(N_CORES)),
                               trace=trace)
    LAST_RESULTS = res
    parts = [np.ascontiguousarray(r["y"].T) for r in res.results]
    out = np.stack([parts[2 * b] + parts[2 * b + 1] + bo[None, :]
                    for b in range(4)])
    return out.astype(np.float32)
